# revision 2
# baseline (speedup 1.0000x reference)
"""Trainium2 Bass kernel for nn_DiamondEmbedding (compositional embedding lookup).

out[b, l, :] = table[(ids[b,l] & Q) % CAP] + table[(ids[b,l] & R) % CAP]
             + table[(ids[b,l] & P) % CAP]

Strategy: batch-shard across the 8 NeuronCores (core c owns batch rows
[c*512, (c+1)*512)); the table is replicated so every lookup is local and no
collectives are needed.  Slot computation (mask + mod) is cheap int math done
on host; the device does the memory-bound part: 76,800 random 512-B row
gathers per core (indirect DMA), a 3-way add on DVE, and contiguous stores.
"""

import sys

if "/opt/trn_rl_repo" not in sys.path:
    sys.path.insert(0, "/opt/trn_rl_repo")

import numpy as np

CAP = 1_000_000
D = 128
B, L = 4096, 50
N_CORES = 8
ROWS_PER_CORE = (B // N_CORES) * L          # 25600
KPP = ROWS_PER_CORE // 128                  # 200 rows per partition per mask
TW = 25                                     # sub-gathers per macro tile
NT = KPP // TW                              # 8 macro tiles

Q_MASK = np.int64(-9223367638808264705)
R_MASK = np.int64(-4398044413953)
P_MASK = np.int64(-2097152)

_compiled = {}


def _build():
    from concourse import bass, bacc, mybir
    import concourse.tile as tile

    nc = bacc.Bacc(
        "TRN2",
        target_bir_lowering=False,
        debug=False,
        enable_asserts=False,
        num_devices=N_CORES,
    )
    slots_t = nc.dram_tensor("slots", [128, 3 * KPP], mybir.dt.int32, kind="ExternalInput")
    table_t = nc.dram_tensor("table", [CAP, D], mybir.dt.float32, kind="ExternalInput")
    out_t = nc.dram_tensor("out", [ROWS_PER_CORE, D], mybir.dt.float32, kind="ExternalOutput")

    with tile.TileContext(nc) as tc:
        with (
            tc.tile_pool(name="slots", bufs=1) as spool,
            tc.tile_pool(name="g", bufs=2) as gpool,
        ):
            sl = spool.tile([128, 3 * KPP], mybir.dt.int32)
            nc.sync.dma_start(sl[:], slots_t.ap())
            out_view = out_t.ap().rearrange("(p k) d -> p (k d)", p=128)
            for t in range(NT):
                gs = [
                    gpool.tile(
                        [128, TW * D],
                        mybir.dt.float32,
                        tag=f"g{m}",
                        name=f"g{m}_{t}",
                    )
                    for m in range(3)
                ]
                for m in range(3):
                    for j in range(TW):
                        col = m * KPP + t * TW + j
                        nc.gpsimd.indirect_dma_start(
                            out=gs[m][:, j * D : (j + 1) * D],
                            out_offset=None,
                            in_=table_t.ap(),
                            in_offset=bass.IndirectOffsetOnAxis(
                                ap=sl[:, col : col + 1], axis=0
                            ),
                        )
                nc.vector.tensor_add(out=gs[0][:], in0=gs[0][:], in1=gs[1][:])
                nc.vector.tensor_add(out=gs[0][:], in0=gs[0][:], in1=gs[2][:])
                nc.sync.dma_start(
                    out_view[:, t * TW * D : (t + 1) * TW * D], gs[0][:]
                )

    nc.compile()
    return nc


def _get_nc():
    if "nc" not in _compiled:
        _compiled["nc"] = _build()
    return _compiled["nc"]


def _make_in_maps(ids, table):
    ids = np.asarray(ids)
    table = np.ascontiguousarray(np.asarray(table, dtype=np.float32))
    masks = np.array([Q_MASK, R_MASK, P_MASK], dtype=np.int64)
    # [3, B, L] -> slot per (mask, b, l); ids are in [0, 2^62) so % is plain mod
    slots = ((ids[None, :, :] & masks[:, None, None]) % CAP).astype(np.int32)
    bpc = B // N_CORES
    in_maps = []
    for c in range(N_CORES):
        s = slots[:, c * bpc : (c + 1) * bpc, :].reshape(3, ROWS_PER_CORE)
        # packed[p, m*KPP + k] = slot for mask m, local output row p*KPP + k
        packed = np.ascontiguousarray(
            s.reshape(3, 128, KPP).transpose(1, 0, 2).reshape(128, 3 * KPP)
        )
        in_maps.append({"slots": packed, "table": table})
    return in_maps


def run_on_hw(ids, table, trace=False):
    """Returns (out [B,L,D] f32, exec_time_ns or None)."""
    from concourse.bass_utils import run_bass_kernel_spmd

    nc = _get_nc()
    in_maps = _make_in_maps(ids, table)
    res = run_bass_kernel_spmd(
        nc, in_maps, core_ids=list(range(N_CORES)), trace=trace
    )
    out = np.concatenate([res.results[c]["out"] for c in range(N_CORES)], axis=0)
    return out.reshape(B, L, D), res.exec_time_ns


def kernel(ids, table):
    out, _ = run_on_hw(ids, table, trace=False)
    return out
